# revision 30
# baseline (speedup 1.0000x reference)
"""Trainium2 Bass kernel for nn_BCNet: three-way low-rank bilinear net.

reference:
  v_ = relu(v @ Wv.T + bv)            # (B, NV, HK)
  q_ = relu(q @ Wq.T + bq)            # (B, NQ, HK)
  logits = einsum('hk,bvk,bqk->bhvq', h_mat, v_, q_) + h_bias

Sharding: data-parallel over batch, 4 batch items per core (8 cores).
All matmuls in bf16 with fp32 PSUM accumulation.

Schedule (evolved from trace analysis; see git-less history in logs):
  - PE runs at peak cadence (~215.7ns per 512-free matmul); all the
    optimization is in (a) reducing PE work and (b) head/tail overlap.
  - Stage A (the 12.9 GF/core v-projection) uses one level of Strassen
    over the (j=1536, k=2048) blocks of Wv: 7 products of 48 free-256
    matmuls instead of 8 block-products -> 36.6us instead of 41.4us per
    batch.  The 5 W-side block-combinations are precomputed on the host
    (free) and streamed per batch through the SBUF buffers that held Wq
    during stage B (tag-shared, zero extra SBUF).  The 5 V-side
    combinations are cheap DVE tensor-tensor ops on vt slices.  Product
    results drain from PSUM into f32 SBUF accumulators (Scalar copy for
    the first contribution, DVE adds after), then per-j-chunk
    ReLU+bias activations produce vact.  Product order (M3,M4,M5,M1,
    M7,M2,M6) completes the C-blocks in the order stage C consumes
    them, so the PE never waits at the A->C boundary.
  - Warmup junk matmuls ride out the PE p-state ramp while the first
    DMAs land; bulk loads all ride ONE queue (cold queues ramp slowly)
    in exact consumption order; stage B is an 8-wide then 4-wide pass
    so early chunk consumption stays above the cold-queue delivery rate.
  - Stage C is head-major: psum [128q x 512v] per (b,h) in two free-256
    n-half chains; h_bias[h] is a per-partition constant so the Scalar
    activation (Identity, bias) does psum->SBUF + bias in one op; the
    host transposes (q,v)->(v,q) at the end.

Host prep per core:
  vT   (4, 2048, 512) bf16  : v[b].T per batch item
  qT   (1024, 512)    bf16  : q[4c:4c+4] transposed+stacked, cols = b*128+q
  WvT  (2048, 1536)   bf16  (only the A11/A22 quadrants are DMA'd)
  cmb  (5, 128, 8, 768) bf16: Strassen W-combos, lhsT layout, stream order
                              [A11+A12, A11+A22, A12-A22, A21+A22, A21-A11]
  WqT  (1024, 1536)   bf16
  bvT  (128, 12) f32 : bv[jc*128+p]
  bqT  (128, 12) f32
  hm   (128, 12, 8) f32 : h_mat[h, jc*128+p]
  hb   (128, 8)  f32 : h_bias[h] broadcast over partitions
Device output per core: out (4, 8, 128, 512) f32 = [b, h, q, v].
Host post: concat -> (32, 8, 128, 512) -> transpose -> (32, 8, 512, 128).
"""

import numpy as np

B, NV, NQ = 32, 512, 128
V_DIM, Q_DIM, HK, H_OUT = 2048, 1024, 1536, 8
N_CORES = 8
BPC = B // N_CORES          # 4 batch items per core
JC = HK // 128              # 12 k-chunks
DCV = V_DIM // 128          # 16 contraction chunks for v
DCQ = Q_DIM // 128          # 8 contraction chunks for q
NH = NV // 2                # 256: Strassen n-half

_CACHE = {}
_DEBUG = False


def _build_nc():
    import concourse.tile as tile
    from concourse import bacc, mybir
    from contextlib import ExitStack

    bf16 = mybir.dt.bfloat16
    f32 = mybir.dt.float32
    ADD = mybir.AluOpType.add
    SUB = mybir.AluOpType.subtract
    MUL = mybir.AluOpType.mult
    Relu = mybir.ActivationFunctionType.Relu
    Ident = mybir.ActivationFunctionType.Identity
    Copy = mybir.ActivationFunctionType.Copy

    nc = bacc.Bacc()

    vT = nc.declare_dram_parameter("vT", [BPC, V_DIM, NV], bf16, isOutput=False)
    qT = nc.declare_dram_parameter("qT", [Q_DIM, BPC * NQ], bf16, isOutput=False)
    WvT = nc.declare_dram_parameter("WvT", [V_DIM, HK], bf16, isOutput=False)
    cmb = nc.declare_dram_parameter("cmb", [5, 128, DCQ, 768], bf16, isOutput=False)
    WqT = nc.declare_dram_parameter("WqT", [Q_DIM, HK], bf16, isOutput=False)
    bvT = nc.declare_dram_parameter("bvT", [128, JC], f32, isOutput=False)
    bqT = nc.declare_dram_parameter("bqT", [128, JC], f32, isOutput=False)
    hm = nc.declare_dram_parameter("hm", [128, JC, H_OUT], f32, isOutput=False)
    hb = nc.declare_dram_parameter("hb", [128, H_OUT], f32, isOutput=False)
    out = nc.declare_dram_parameter("out", [BPC, H_OUT, NQ, NV], f32, isOutput=True)
    if _DEBUG:
        dbg_vact = nc.declare_dram_parameter(
            "dbg_vact", [BPC, 128, JC, NV], bf16, isOutput=True)
        dbg_qh = nc.declare_dram_parameter(
            "dbg_qh", [BPC, 128, JC, H_OUT * NQ], bf16, isOutput=True)
        dbg_qact = nc.declare_dram_parameter(
            "dbg_qact", [128, JC, BPC * NQ], bf16, isOutput=True)
        dbg_cb = nc.declare_dram_parameter(
            "dbg_cb", [BPC, 4, 128, 6, 256], f32, isOutput=True)

    with ExitStack() as ctx:
        tc = ctx.enter_context(tile.TileContext(nc))
        consts = ctx.enter_context(tc.tile_pool(name="consts", bufs=1))
        qpool = ctx.enter_context(tc.tile_pool(name="qpool", bufs=1))
        vin = ctx.enter_context(tc.tile_pool(name="vin", bufs=2))
        vact = ctx.enter_context(tc.tile_pool(name="vact", bufs=2))
        qhp = ctx.enter_context(tc.tile_pool(name="qhp", bufs=1))
        junkp = ctx.enter_context(tc.tile_pool(name="junkp", bufs=1))
        tcomb = ctx.enter_context(tc.tile_pool(name="tcomb", bufs=3))
        csb = ctx.enter_context(tc.tile_pool(name="csb", bufs=4))
        outp = ctx.enter_context(tc.tile_pool(name="outp", bufs=3))
        psM = ctx.enter_context(tc.tile_pool(name="psM", bufs=2, space="PSUM"))
        psC = ctx.enter_context(tc.tile_pool(name="psC", bufs=2, space="PSUM"))

        # ---- SBUF destination tiles ----
        qt_sb = qpool.tile([128, DCQ, BPC * NQ], bf16)
        # wq lives in two [128,8,1024] buffers (tag "wqbig"); the Strassen
        # W-combos later rotate through the same two buffers.
        wq1_sb = consts.tile([128, DCQ, 1024], bf16, tag="wqbig", bufs=2,
                             name="wq1")
        wq2_sb = consts.tile([128, DCQ, 1024], bf16, tag="wqbig", bufs=2,
                             name="wq2")
        wv11_sb = consts.tile([128, DCQ, 768], bf16, name="wv11")
        wv22_sb = consts.tile([128, DCQ, 768], bf16, name="wv22")
        bq_sb = consts.tile([128, JC], f32)
        bv_sb = consts.tile([128, JC], f32)
        hm_sb = consts.tile([128, JC, H_OUT], f32)
        hb_sb = consts.tile([128, H_OUT], f32)
        vt0_sb = vin.tile([128, DCV, NV], bf16, tag="vt", name="vt0")

        qT_r = qT.rearrange("(d p) n -> p d n", p=128)
        WqT_r = WqT.rearrange("(d p) j -> p d j", p=128)
        WvT_r = WvT.rearrange("(d p) j -> p d j", p=128)
        vT0_r = vT[0].rearrange("(d p) n -> p d n", p=128)

        # ---- warmup: junk matmuls to ride out the PE p-state ramp while
        # the first DMAs land.
        junk = junkp.tile([128, NV], bf16)
        nc.vector.memset(junk, 0.0)
        ps_junk = psC.tile([128, NV], f32, tag="psC", name="ps_junk")
        for w in range(13):
            nc.tensor.matmul(
                ps_junk[:, 0:256], lhsT=junk[:, 0:128], rhs=junk[:, 0:256],
                start=(w == 0), stop=(w == 12),
            )

        # ---- input DMAs: tiny consts on GpSimd; all bulk loads on the
        # single Sync queue in exact consumption order.
        nc.gpsimd.dma_start(out=bq_sb, in_=bqT[:, :])
        nc.gpsimd.dma_start(out=hm_sb, in_=hm[:, :, :])
        nc.gpsimd.dma_start(out=bv_sb, in_=bvT[:, :])
        nc.gpsimd.dma_start(out=hb_sb, in_=hb[:, :])

        for d in range(DCQ):
            nc.sync.dma_start(out=qt_sb[:, d, :], in_=qT_r[:, d, :])
            nc.sync.dma_start(out=wq1_sb[:, d, :], in_=WqT_r[:, d, 0:1024])
        for d in range(DCQ):
            nc.sync.dma_start(out=wq2_sb[:, d, 0:512],
                              in_=WqT_r[:, d, 1024:HK])
        # vt0 fully (the first V-side combos need both k-halves)
        for g in range(4):
            nc.sync.dma_start(out=vt0_sb[:, 4 * g:4 * g + 4, :],
                              in_=vT0_r[:, 4 * g:4 * g + 4, :])
        # A11 / A22 quadrants of Wv, chunked for cold-queue pacing
        for d in range(DCQ):
            nc.sync.dma_start(out=wv11_sb[:, d, :], in_=WvT_r[:, d, 0:768])
        for d in range(DCQ):
            nc.sync.dma_start(out=wv22_sb[:, d, :],
                              in_=WvT_r[:, DCQ + d, 768:HK])

        # ---- stage B: q_ = relu(q @ Wq.T + bq), all 4 b at once ----
        # 8-wide pass1 / 4-wide pass2; chains live in [128,6,256] psM tiles
        # (3 x 512-wide slice-chains each) plus borrowed psC banks.
        qact_sb = qpool.tile([128, JC, BPC * NQ], bf16)

        def b_chain_targets(npsm, npsc, tagp):
            """Chain targets as (tile, kind, idx): psM tiles host 3
            512-wide slice-chains each; psC tiles host one."""
            targets = []
            for t in range(npsm):
                m = psM.tile([128, 6, NH], f32, tag="psM", name=f"{tagp}m{t}")
                for c in range(3):
                    targets.append((m, "m", c))
            for t in range(npsc):
                p = psC.tile([128, NV], f32, tag="psC", name=f"{tagp}c{t}")
                targets.append((p, "p", 0))
            return targets

        def chain_ap(tgt):
            tile_, kind, c = tgt
            return tile_[:, 2 * c:2 * c + 2, :] if kind == "m" else tile_[:, :]

        def half_ap(tgt, hf):
            tile_, kind, c = tgt
            if kind == "m":
                return tile_[:, 2 * c + hf, :]
            return tile_[:, hf * NH:(hf + 1) * NH]

        def w_slice(j, d):
            if j < 8:
                return wq1_sb[:, d, j * 128:(j + 1) * 128]
            return wq2_sb[:, d, (j - 8) * 128:(j - 8 + 1) * 128]

        def run_b_pass(j0, nj, targets):
            for d in range(DCQ - 2):
                for i in range(nj):
                    nc.tensor.matmul(
                        chain_ap(targets[i]), lhsT=w_slice(j0 + i, d),
                        rhs=qt_sb[:, d, :], start=(d == 0), stop=False,
                    )
            for i in range(nj):
                j = j0 + i
                for d in (DCQ - 2, DCQ - 1):
                    nc.tensor.matmul(
                        chain_ap(targets[i]), lhsT=w_slice(j, d),
                        rhs=qt_sb[:, d, :], start=False, stop=(d == DCQ - 1),
                    )
                for hf in range(2):
                    nc.scalar.activation(
                        out=qact_sb[:, j, hf * NH:(hf + 1) * NH],
                        in_=half_ap(targets[i], hf),
                        func=Relu, bias=bq_sb[:, j:j + 1], scale=1.0,
                    )

        run_b_pass(0, 8, b_chain_targets(2, 2, "psB1"))
        run_b_pass(8, 4, b_chain_targets(1, 1, "psB2"))
        if _DEBUG:
            nc.scalar.dma_start(out=dbg_qact[:, :, :], in_=qact_sb[:, :, :])

        # ---- per-batch: Strassen stage A, qh build, stage C ----
        # Strassen blocks (j x k): A11=Wv[0:768,0:1024] etc.
        # products in completion order needed by stage C:
        #   P1 M3 = A11 (B12-B22)      -> C12 += , C22 +=
        #   P2 M4 = A22 (B21-B11)      -> C11 += , C21 +=
        #   P3 M5 = (A11+A12) B22      -> C12 done, C11 -=
        #   P4 M1 = (A11+A22)(B11+B22) -> C11 += , C22 +=
        #   P5 M7 = (A12-A22)(B21+B22) -> C11 done
        #   P6 M2 = (A21+A22) B11      -> C21 done, C22 -=
        #   P7 M6 = (A21-A11)(B11+B12) -> C22 done
        # C11 -> vact[:,0:6,0:256], C12 -> vact[:,0:6,256:512],
        # C21 -> vact[:,6:12,0:256], C22 -> vact[:,6:12,256:512]
        for b in range(BPC):
            if b > 0:
                vt_sb = vin.tile([128, DCV, NV], bf16, tag="vt")
                vT_r = vT[b].rearrange("(d p) n -> p d n", p=128)
                nc.sync.dma_start(out=vt_sb[:, :, :], in_=vT_r[:, :, :])
            else:
                vt_sb = vt0_sb

            # stream this batch's W-combos through the wqbig buffers
            cmb_t = []
            for i in range(5):
                t = consts.tile([128, DCQ, 1024], bf16, tag="wqbig", bufs=2,
                                name=f"cmb{b}_{i}")
                nc.sync.dma_start(out=t[:, :, 0:768], in_=cmb[i][:, :, :])
                cmb_t.append(t)
            c3, c1, c5, c2, c4 = cmb_t

            # B-side slices / combos
            B11 = vt_sb[:, 0:8, 0:NH]
            B12 = vt_sb[:, 0:8, NH:NV]
            B21 = vt_sb[:, 8:16, 0:NH]
            B22 = vt_sb[:, 8:16, NH:NV]

            def tt_tile(name, src0, src1, op):
                t = tcomb.tile([128, DCQ, NH], bf16, tag="tc", name=name)
                nc.vector.tensor_tensor(out=t, in0=src0, in1=src1, op=op)
                return t

            t3 = tt_tile(f"t3_{b}", B12, B22, SUB)
            t4 = tt_tile(f"t4_{b}", B21, B11, SUB)

            vact_sb = vact.tile([128, JC, NV], bf16, tag="vact")
            c_blocks = {}
            for nm in ("C11", "C12", "C21", "C22"):
                c_blocks[nm] = csb.tile([128, 6, NH], f32, tag="csb",
                                        name=f"{nm}_{b}")

            def product(lhs_sb, lhs_off, rhs, actions, relu=None):
                """One Strassen product.  Chains run in bank-interleaved
                order [0,2,4,1,3,5] and each chunk is drained (actions on
                c_blocks, then optional relu into vact) immediately after
                its chain stops -- a later chain's start resets its whole
                PSUM bank, so the bank-mate's data must be consumed first.
                actions: list of (dst, op) with op in ('copy', ADD, SUB)."""
                pm = psM.tile([128, 6, NH], f32, tag="psM")
                for jc in (0, 2, 4, 1, 3, 5):
                    for d in range(DCQ):
                        nc.tensor.matmul(
                            pm[:, jc, :],
                            lhsT=lhs_sb[:, d, lhs_off + jc * 128:
                                        lhs_off + (jc + 1) * 128],
                            rhs=rhs[:, d, :],
                            start=(d == 0), stop=(d == DCQ - 1),
                        )
                    for ai, (dst, op) in enumerate(actions):
                        cb = c_blocks[dst]
                        if op == "copy":
                            nc.scalar.activation(out=cb[:, jc, :],
                                                 in_=pm[:, jc, :], func=Copy)
                        else:
                            nc.vector.tensor_tensor(
                                out=cb[:, jc, :], in0=cb[:, jc, :],
                                in1=pm[:, jc, :], op=op)
                        if ai == 0 and relu is not None:
                            nm_, j0_, nsl_ = relu
                            j = j0_ + jc
                            nc.scalar.activation(
                                out=vact_sb[:, j, nsl_],
                                in_=c_blocks[nm_][:, jc, :],
                                func=Relu, bias=bv_sb[:, j:j + 1], scale=1.0,
                            )

            n1 = slice(0, NH)
            n2 = slice(NH, NV)

            product(wv11_sb, 0, t3, [("C12", "copy"), ("C22", "copy")])  # M3
            product(wv22_sb, 0, t4, [("C11", "copy"), ("C21", "copy")])  # M4
            t1 = tt_tile(f"t1_{b}", B11, B22, ADD)
            product(c3, 0, B22, [("C12", ADD), ("C11", SUB)],
                    relu=("C12", 0, n2))                                 # M5
            t7 = tt_tile(f"t7_{b}", B21, B22, ADD)
            product(c1, 0, t1, [("C11", ADD), ("C22", ADD)])             # M1
            t6 = tt_tile(f"t6_{b}", B11, B12, ADD)
            product(c5, 0, t7, [("C11", ADD)], relu=("C11", 0, n1))      # M7
            product(c2, 0, B11, [("C21", ADD), ("C22", SUB)],
                    relu=("C21", 6, n1))                                 # M2
            product(c4, 0, t6, [("C22", ADD)], relu=("C22", 6, n2))      # M6
            if _DEBUG:
                for i_, nm_ in enumerate(("C11", "C12", "C21", "C22")):
                    nc.gpsimd.dma_start(out=dbg_cb[b][i_][:, :, :],
                                        in_=c_blocks[nm_][:, :, :])
                nc.scalar.dma_start(out=dbg_vact[b][:, :, :],
                                    in_=vact_sb[:, :, :])

            # ---- build Qh[b][k, h*128+q'] = q_[k, b*128+q'] * h_mat[h, k]
            # one broadcast tensor-tensor per head
            qh_sb = qhp.tile([128, JC, H_OUT * NQ], bf16, tag="qh")
            qa = qact_sb[:, :, b * NQ:(b + 1) * NQ]
            for h in range(H_OUT):
                nc.vector.tensor_tensor(
                    out=qh_sb[:, :, h * NQ:(h + 1) * NQ],
                    in0=qa,
                    in1=hm_sb[:, :, h].broadcast_to([128, JC, NQ]),
                    op=MUL,
                )
            if _DEBUG:
                nc.scalar.dma_start(out=dbg_qh[b][:, :, :], in_=qh_sb[:, :, :])

            # ---- stage C: logits[b,h] = (qh_h).T @ v_ -> psum [128q, 512v]
            for h in range(H_OUT):
                last = (b == BPC - 1 and h == H_OUT - 1)
                if not last:
                    po = psC.tile([128, NV], f32, tag="psC")
                    for j in range(JC):
                        nc.tensor.matmul(
                            po,
                            lhsT=qh_sb[:, j, h * NQ:(h + 1) * NQ],
                            rhs=vact_sb[:, j, :],
                            start=(j == 0), stop=(j == JC - 1),
                        )
                    o_sb = outp.tile([128, NV], f32, tag="osb")
                    nc.scalar.activation(
                        out=o_sb, in_=po,
                        func=Ident, bias=hb_sb[:, h:h + 1], scale=1.0,
                    )
                    eng = nc.gpsimd if h % 2 == 0 else nc.sync
                    eng.dma_start(out=out[b, h, :, :], in_=o_sb)
                else:
                    # last output: two free-256 chains in separate psC tiles
                    # so the first half's act+store launches early and the
                    # final store is only 128KB
                    engs = (nc.gpsimd, nc.sync)
                    for half in range(2):
                        sl = slice(half * NH, (half + 1) * NH)
                        po = psC.tile([128, NH], f32, tag="psC",
                                      name=f"psC_last{half}")
                        for j in range(JC):
                            nc.tensor.matmul(
                                po,
                                lhsT=qh_sb[:, j, h * NQ:(h + 1) * NQ],
                                rhs=vact_sb[:, j, sl],
                                start=(j == 0), stop=(j == JC - 1),
                            )
                        o_sb = outp.tile([128, NH], f32, tag="osb",
                                         name=f"osb_last{half}")
                        nc.scalar.activation(
                            out=o_sb, in_=po,
                            func=Ident, bias=hb_sb[:, h:h + 1], scale=1.0,
                        )
                        engs[half].dma_start(out=out[b, h, :, sl], in_=o_sb)

    nc.compile()
    return nc


def kernel(v, q, Wv, bv, Wq, bq, h_mat, h_bias):
    import ml_dtypes
    from concourse import bass_utils

    bf16 = ml_dtypes.bfloat16

    if "nc" not in _CACHE:
        _CACHE["nc"] = _build_nc()
    nc = _CACHE["nc"]

    v = np.asarray(v, dtype=np.float32)
    q = np.asarray(q, dtype=np.float32)
    Wv = np.asarray(Wv, dtype=np.float32)
    Wq = np.asarray(Wq, dtype=np.float32)
    bv = np.asarray(bv, dtype=np.float32)
    bq = np.asarray(bq, dtype=np.float32)
    h_mat = np.asarray(h_mat, dtype=np.float32)
    h_bias = np.asarray(h_bias, dtype=np.float32)

    vT = np.ascontiguousarray(v.transpose(0, 2, 1)).astype(bf16)      # (B, 2048, 512)
    WvT_f = np.ascontiguousarray(Wv.T)                                # (2048, 1536) f32
    WvT = WvT_f.astype(bf16)
    WqT = np.ascontiguousarray(Wq.T).astype(bf16)                     # (1024, 1536)
    bvT = np.ascontiguousarray(bv.reshape(JC, 128).T)                 # (128, 12)
    bqT = np.ascontiguousarray(bq.reshape(JC, 128).T)
    hmP = np.ascontiguousarray(h_mat.reshape(H_OUT, JC, 128).transpose(2, 1, 0))
    hbB = np.ascontiguousarray(np.broadcast_to(h_bias[None, :], (128, H_OUT)))

    # Strassen W-combos in lhsT layout [128, 8, 768], stream order
    # [A11+A12, A11+A22, A12-A22, A21+A22, A21-A11]
    # (A11=W[0:768,0:1024] -> WvT[0:1024, 0:768] etc.)
    T = WvT_f
    combos = [
        T[0:1024, 0:768] + T[1024:2048, 0:768],     # A11+A12  (c3, M5)
        T[0:1024, 0:768] + T[1024:2048, 768:1536],  # A11+A22  (c1, M1)
        T[1024:2048, 0:768] - T[1024:2048, 768:1536],  # A12-A22 (c5, M7)
        T[0:1024, 768:1536] + T[1024:2048, 768:1536],  # A21+A22 (c2, M2)
        T[0:1024, 768:1536] - T[0:1024, 0:768],     # A21-A11  (c4, M6)
    ]
    cmbA = np.stack([
        np.ascontiguousarray(
            c.reshape(DCQ, 128, 768).transpose(1, 0, 2)).astype(bf16)
        for c in combos
    ])  # (5, 128, 8, 768)

    in_maps = []
    for c in range(N_CORES):
        bs = slice(BPC * c, BPC * (c + 1))
        qTc = np.ascontiguousarray(
            q[bs].transpose(2, 0, 1).reshape(Q_DIM, BPC * NQ)
        ).astype(bf16)
        in_maps.append({
            "vT": vT[bs],
            "qT": qTc,
            "WvT": WvT,
            "cmb": cmbA,
            "WqT": WqT,
            "bvT": bvT,
            "bqT": bqT,
            "hm": hmP,
            "hb": hbB,
        })

    res = bass_utils.run_bass_kernel_spmd(nc, in_maps, list(range(N_CORES)))
    outs = np.concatenate([res.results[c]["out"] for c in range(N_CORES)], axis=0)
    # (32, 8, 128, 512) -> (32, 8, 512, 128)
    logits = outs.transpose(0, 1, 3, 2)
    return np.ascontiguousarray(logits)


# revision 31
# speedup vs baseline: 1.1389x; 1.1389x over previous
"""Trainium2 Bass kernel for nn_BCNet: three-way low-rank bilinear net.

reference:
  v_ = relu(v @ Wv.T + bv)            # (B, NV, HK)
  q_ = relu(q @ Wq.T + bq)            # (B, NQ, HK)
  logits = einsum('hk,bvk,bqk->bhvq', h_mat, v_, q_) + h_bias

Sharding: data-parallel over batch, 4 batch items per core (8 cores).
All matmuls in bf16 with fp32 PSUM accumulation.

Schedule (evolved from trace analysis; see git-less history in logs):
  - PE runs at peak cadence (~215.7ns per 512-free matmul); all the
    optimization is in (a) reducing PE work and (b) head/tail overlap.
  - Stage A (the 12.9 GF/core v-projection) uses one level of Strassen
    over the (j=1536, k=2048) blocks of Wv: 7 products of 48 free-256
    matmuls instead of 8 block-products -> 36.6us instead of 41.4us per
    batch.  The 5 W-side block-combinations are precomputed on the host
    (free) and streamed per batch through the SBUF buffers that held Wq
    during stage B (tag-shared, zero extra SBUF).  The 5 V-side
    combinations are cheap DVE tensor-tensor ops on vt slices.  Product
    results drain from PSUM into f32 SBUF accumulators (Scalar copy for
    the first contribution, DVE adds after), then per-j-chunk
    ReLU+bias activations produce vact.  Product order (M3,M4,M5,M1,
    M7,M2,M6) completes the C-blocks in the order stage C consumes
    them, so the PE never waits at the A->C boundary.
  - Warmup junk matmuls ride out the PE p-state ramp while the first
    DMAs land; bulk loads all ride ONE queue (cold queues ramp slowly)
    in exact consumption order; stage B is an 8-wide then 4-wide pass
    so early chunk consumption stays above the cold-queue delivery rate.
  - Stage C is head-major: psum [128q x 512v] per (b,h) in two free-256
    n-half chains; h_bias[h] is a per-partition constant so the Scalar
    activation (Identity, bias) does psum->SBUF + bias in one op; the
    host transposes (q,v)->(v,q) at the end.

Host prep per core:
  vT   (4, 2048, 512) bf16  : v[b].T per batch item
  qT   (1024, 512)    bf16  : q[4c:4c+4] transposed+stacked, cols = b*128+q
  WvT  (2048, 1536)   bf16  (only the A11/A22 quadrants are DMA'd)
  cmb  (5, 128, 8, 768) bf16: Strassen W-combos, lhsT layout, stream order
                              [A11+A12, A11+A22, A12-A22, A21+A22, A21-A11]
  WqT  (1024, 1536)   bf16
  bvT  (128, 12) f32 : bv[jc*128+p]
  bqT  (128, 12) f32
  hm   (128, 12, 8) f32 : h_mat[h, jc*128+p]
  hb   (128, 8)  f32 : h_bias[h] broadcast over partitions
Device output per core: out (4, 8, 128, 512) f32 = [b, h, q, v].
Host post: concat -> (32, 8, 128, 512) -> transpose -> (32, 8, 512, 128).
"""

import numpy as np

B, NV, NQ = 32, 512, 128
V_DIM, Q_DIM, HK, H_OUT = 2048, 1024, 1536, 8
N_CORES = 8
BPC = B // N_CORES          # 4 batch items per core
JC = HK // 128              # 12 k-chunks
DCV = V_DIM // 128          # 16 contraction chunks for v
DCQ = Q_DIM // 128          # 8 contraction chunks for q
NH = NV // 2                # 256: Strassen n-half

_CACHE = {}
_DEBUG = False


def _build_nc():
    import concourse.tile as tile
    from concourse import bacc, mybir
    from contextlib import ExitStack

    bf16 = mybir.dt.bfloat16
    f32 = mybir.dt.float32
    ADD = mybir.AluOpType.add
    SUB = mybir.AluOpType.subtract
    MUL = mybir.AluOpType.mult
    Relu = mybir.ActivationFunctionType.Relu
    Ident = mybir.ActivationFunctionType.Identity
    Copy = mybir.ActivationFunctionType.Copy

    nc = bacc.Bacc()

    vT = nc.declare_dram_parameter("vT", [BPC, V_DIM, NV], bf16, isOutput=False)
    qT = nc.declare_dram_parameter("qT", [Q_DIM, BPC * NQ], bf16, isOutput=False)
    WvT = nc.declare_dram_parameter("WvT", [V_DIM, HK], bf16, isOutput=False)
    cmb = nc.declare_dram_parameter("cmb", [5, 128, DCQ, 768], bf16, isOutput=False)
    WqT = nc.declare_dram_parameter("WqT", [Q_DIM, HK], bf16, isOutput=False)
    bvT = nc.declare_dram_parameter("bvT", [128, JC], f32, isOutput=False)
    bqT = nc.declare_dram_parameter("bqT", [128, JC], f32, isOutput=False)
    hm = nc.declare_dram_parameter("hm", [128, JC, H_OUT], f32, isOutput=False)
    hb = nc.declare_dram_parameter("hb", [128, H_OUT], f32, isOutput=False)
    out = nc.declare_dram_parameter("out", [BPC, H_OUT, NQ, NV], f32, isOutput=True)
    if _DEBUG:
        dbg_vact = nc.declare_dram_parameter(
            "dbg_vact", [BPC, 128, JC, NV], bf16, isOutput=True)
        dbg_qh = nc.declare_dram_parameter(
            "dbg_qh", [BPC, 128, JC, H_OUT * NQ], bf16, isOutput=True)
        dbg_qact = nc.declare_dram_parameter(
            "dbg_qact", [128, JC, BPC * NQ], bf16, isOutput=True)
        dbg_cb = nc.declare_dram_parameter(
            "dbg_cb", [BPC, 4, 128, 6, 256], f32, isOutput=True)

    with ExitStack() as ctx:
        tc = ctx.enter_context(tile.TileContext(nc))
        consts = ctx.enter_context(tc.tile_pool(name="consts", bufs=1))
        qpool = ctx.enter_context(tc.tile_pool(name="qpool", bufs=1))
        vin = ctx.enter_context(tc.tile_pool(name="vin", bufs=2))
        vact = ctx.enter_context(tc.tile_pool(name="vact", bufs=2))
        qhp = ctx.enter_context(tc.tile_pool(name="qhp", bufs=1))
        junkp = ctx.enter_context(tc.tile_pool(name="junkp", bufs=1))
        tcomb = ctx.enter_context(tc.tile_pool(name="tcomb", bufs=3))
        csb = ctx.enter_context(tc.tile_pool(name="csb", bufs=4))
        outp = ctx.enter_context(tc.tile_pool(name="outp", bufs=3))
        psM = ctx.enter_context(tc.tile_pool(name="psM", bufs=2, space="PSUM"))
        psC = ctx.enter_context(tc.tile_pool(name="psC", bufs=2, space="PSUM"))

        # ---- SBUF destination tiles ----
        qt_sb = qpool.tile([128, DCQ, BPC * NQ], bf16)
        # wq lives in two [128,8,1024] buffers (tag "wqbig"); the Strassen
        # W-combos later rotate through the same two buffers.
        wq1_sb = consts.tile([128, DCQ, 1024], bf16, tag="wqbig", bufs=2,
                             name="wq1")
        wq2_sb = consts.tile([128, DCQ, 1024], bf16, tag="wqbig", bufs=2,
                             name="wq2")
        wv11_sb = consts.tile([128, DCQ, 768], bf16, name="wv11")
        wv22_sb = consts.tile([128, DCQ, 768], bf16, name="wv22")
        bq_sb = consts.tile([128, JC], f32)
        bv_sb = consts.tile([128, JC], f32)
        hm_sb = consts.tile([128, JC, H_OUT], f32)
        hb_sb = consts.tile([128, H_OUT], f32)
        vt0_sb = vin.tile([128, DCV, NV], bf16, tag="vt", name="vt0")

        qT_r = qT.rearrange("(d p) n -> p d n", p=128)
        WqT_r = WqT.rearrange("(d p) j -> p d j", p=128)
        WvT_r = WvT.rearrange("(d p) j -> p d j", p=128)
        vT0_r = vT[0].rearrange("(d p) n -> p d n", p=128)

        # ---- warmup: junk matmuls to ride out the PE p-state ramp while
        # the first DMAs land.
        junk = junkp.tile([128, NV], bf16)
        nc.vector.memset(junk, 0.0)
        zeros_sb = junkp.tile([128, 6, NH], bf16, name="zeros")
        nc.vector.memset(zeros_sb, 0.0)

        def zero_psum(pm):
            # DVE 0+0 write: zeroes psum without reading (possibly NaN)
            # uninitialized content and without matmul start's bank reset
            nc.vector.tensor_tensor(out=pm, in0=zeros_sb, in1=zeros_sb,
                                    op=ADD)

        ps_junk = psC.tile([128, NV], f32, tag="psC", name="ps_junk")
        for w in range(13):
            nc.tensor.matmul(
                ps_junk[:, 0:256], lhsT=junk[:, 0:128], rhs=junk[:, 0:256],
                start=(w == 0), stop=(w == 12),
            )

        # ---- input DMAs: tiny consts on GpSimd; all bulk loads on the
        # single Sync queue in exact consumption order.
        nc.gpsimd.dma_start(out=bq_sb, in_=bqT[:, :])
        nc.gpsimd.dma_start(out=hm_sb, in_=hm[:, :, :])
        nc.gpsimd.dma_start(out=bv_sb, in_=bvT[:, :])
        nc.gpsimd.dma_start(out=hb_sb, in_=hb[:, :])

        for d in range(DCQ):
            nc.sync.dma_start(out=qt_sb[:, d, :], in_=qT_r[:, d, :])
            nc.sync.dma_start(out=wq1_sb[:, d, :], in_=WqT_r[:, d, 0:1024])
        for d in range(DCQ):
            nc.sync.dma_start(out=wq2_sb[:, d, 0:512],
                              in_=WqT_r[:, d, 1024:HK])
        # vt0 fully (the first V-side combos need both k-halves)
        for g in range(4):
            nc.sync.dma_start(out=vt0_sb[:, 4 * g:4 * g + 4, :],
                              in_=vT0_r[:, 4 * g:4 * g + 4, :])
        # A11 / A22 quadrants of Wv, chunked for cold-queue pacing
        for d in range(DCQ):
            nc.sync.dma_start(out=wv11_sb[:, d, :], in_=WvT_r[:, d, 0:768])
        for d in range(DCQ):
            nc.sync.dma_start(out=wv22_sb[:, d, :],
                              in_=WvT_r[:, DCQ + d, 768:HK])

        # ---- stage B: q_ = relu(q @ Wq.T + bq), all 4 b at once ----
        # 8-wide pass1 / 4-wide pass2; chains live in [128,6,256] psM tiles
        # (3 x 512-wide slice-chains each) plus borrowed psC banks.
        qact_sb = qpool.tile([128, JC, BPC * NQ], bf16)

        def b_chain_targets(npsm, npsc, tagp):
            """Chain targets as (tile, kind, idx): psM tiles host 3
            512-wide slice-chains each; psC tiles host one."""
            targets = []
            for t in range(npsm):
                m = psM.tile([128, 6, NH], f32, tag="psM", name=f"{tagp}m{t}")
                for c in range(3):
                    targets.append((m, "m", c))
            for t in range(npsc):
                p = psC.tile([128, NV], f32, tag="psC", name=f"{tagp}c{t}")
                targets.append((p, "p", 0))
            return targets

        def chain_ap(tgt):
            tile_, kind, c = tgt
            return tile_[:, 2 * c:2 * c + 2, :] if kind == "m" else tile_[:, :]

        def half_ap(tgt, hf):
            tile_, kind, c = tgt
            if kind == "m":
                return tile_[:, 2 * c + hf, :]
            return tile_[:, hf * NH:(hf + 1) * NH]

        def w_slice(j, d):
            if j < 8:
                return wq1_sb[:, d, j * 128:(j + 1) * 128]
            return wq2_sb[:, d, (j - 8) * 128:(j - 8 + 1) * 128]

        def run_b_pass(j0, nj, targets):
            for d in range(DCQ - 2):
                for i in range(nj):
                    nc.tensor.matmul(
                        chain_ap(targets[i]), lhsT=w_slice(j0 + i, d),
                        rhs=qt_sb[:, d, :], start=(d == 0), stop=False,
                    )
            for i in range(nj):
                j = j0 + i
                for d in (DCQ - 2, DCQ - 1):
                    nc.tensor.matmul(
                        chain_ap(targets[i]), lhsT=w_slice(j, d),
                        rhs=qt_sb[:, d, :], start=False, stop=(d == DCQ - 1),
                    )
                for hf in range(2):
                    nc.scalar.activation(
                        out=qact_sb[:, j, hf * NH:(hf + 1) * NH],
                        in_=half_ap(targets[i], hf),
                        func=Relu, bias=bq_sb[:, j:j + 1], scale=1.0,
                    )

        run_b_pass(0, 8, b_chain_targets(2, 2, "psB1"))
        run_b_pass(8, 4, b_chain_targets(1, 1, "psB2"))
        if _DEBUG:
            nc.scalar.dma_start(out=dbg_qact[:, :, :], in_=qact_sb[:, :, :])

        # ---- per-batch: Strassen stage A, qh build, stage C ----
        # Strassen blocks (j x k): A11=Wv[0:768,0:1024] etc.
        # products in completion order needed by stage C:
        #   P1 M3 = A11 (B12-B22)      -> C12 += , C22 +=
        #   P2 M4 = A22 (B21-B11)      -> C11 += , C21 +=
        #   P3 M5 = (A11+A12) B22      -> C12 done, C11 -=
        #   P4 M1 = (A11+A22)(B11+B22) -> C11 += , C22 +=
        #   P5 M7 = (A12-A22)(B21+B22) -> C11 done
        #   P6 M2 = (A21+A22) B11      -> C21 done, C22 -=
        #   P7 M6 = (A21-A11)(B11+B12) -> C22 done
        # C11 -> vact[:,0:6,0:256], C12 -> vact[:,0:6,256:512],
        # C21 -> vact[:,6:12,0:256], C22 -> vact[:,6:12,256:512]
        for b in range(BPC):
            if b > 0:
                vt_sb = vin.tile([128, DCV, NV], bf16, tag="vt")
                vT_r = vT[b].rearrange("(d p) n -> p d n", p=128)
                nc.sync.dma_start(out=vt_sb[:, :, :], in_=vT_r[:, :, :])
            else:
                vt_sb = vt0_sb

            # stream this batch's W-combos through the wqbig buffers
            cmb_t = []
            for i in range(5):
                t = consts.tile([128, DCQ, 1024], bf16, tag="wqbig", bufs=2,
                                name=f"cmb{b}_{i}")
                nc.sync.dma_start(out=t[:, :, 0:768], in_=cmb[i][:, :, :])
                cmb_t.append(t)
            c3, c1, c5, c2, c4 = cmb_t

            # B-side slices / combos
            B11 = vt_sb[:, 0:8, 0:NH]
            B12 = vt_sb[:, 0:8, NH:NV]
            B21 = vt_sb[:, 8:16, 0:NH]
            B22 = vt_sb[:, 8:16, NH:NV]

            def tt_tile(name, src0, src1, op):
                t = tcomb.tile([128, DCQ, NH], bf16, tag="tc", name=name)
                nc.vector.tensor_tensor(out=t, in0=src0, in1=src1, op=op)
                return t

            t3 = tt_tile(f"t3_{b}", B12, B22, SUB)
            t4 = tt_tile(f"t4_{b}", B21, B11, SUB)

            vact_sb = vact.tile([128, JC, NV], bf16, tag="vact")
            c_blocks = {}
            for nm in ("C11", "C12", "C21", "C22"):
                c_blocks[nm] = csb.tile([128, 6, NH], f32, tag="csb",
                                        name=f"{nm}_{b}")

            def product(lhs_sb, lhs_off, rhs, actions, relu=None):
                """One Strassen product.  The psum tile is zeroed once and
                all chains accumulate with start=False (start resets whole
                PSUM banks, which corrupts bank-sharing neighbours), so
                drains are single coarse ops per action.
                actions: list of (dst, op) with op in ('copy', ADD, SUB)."""
                pm = psM.tile([128, 6, NH], f32, tag="psM")
                zero_psum(pm)
                for d in range(DCQ):
                    for jc in range(6):
                        nc.tensor.matmul(
                            pm[:, jc, :],
                            lhsT=lhs_sb[:, d, lhs_off + jc * 128:
                                        lhs_off + (jc + 1) * 128],
                            rhs=rhs[:, d, :],
                            start=False, stop=(d == DCQ - 1),
                            skip_group_check=True,
                        )
                for dst, op in actions:
                    cb = c_blocks[dst]
                    if op == "copy":
                        nc.scalar.activation(out=cb, in_=pm, func=Copy)
                    else:
                        nc.vector.tensor_tensor(out=cb, in0=cb, in1=pm, op=op)
                if relu is not None:
                    nm_, j0_, nsl_ = relu
                    for jc in range(6):
                        j = j0_ + jc
                        nc.scalar.activation(
                            out=vact_sb[:, j, nsl_],
                            in_=c_blocks[nm_][:, jc, :],
                            func=Relu, bias=bv_sb[:, j:j + 1], scale=1.0,
                        )

            n1 = slice(0, NH)
            n2 = slice(NH, NV)

            product(wv11_sb, 0, t3, [("C12", "copy"), ("C22", "copy")])  # M3
            product(wv22_sb, 0, t4, [("C11", "copy"), ("C21", "copy")])  # M4
            t1 = tt_tile(f"t1_{b}", B11, B22, ADD)
            product(c3, 0, B22, [("C12", ADD), ("C11", SUB)],
                    relu=("C12", 0, n2))                                 # M5
            t7 = tt_tile(f"t7_{b}", B21, B22, ADD)
            product(c1, 0, t1, [("C11", ADD), ("C22", ADD)])             # M1
            t6 = tt_tile(f"t6_{b}", B11, B12, ADD)
            product(c5, 0, t7, [("C11", ADD)], relu=("C11", 0, n1))      # M7
            product(c2, 0, B11, [("C21", ADD), ("C22", SUB)],
                    relu=("C21", 6, n1))                                 # M2
            product(c4, 0, t6, [("C22", ADD)], relu=("C22", 6, n2))      # M6
            if _DEBUG:
                for i_, nm_ in enumerate(("C11", "C12", "C21", "C22")):
                    nc.gpsimd.dma_start(out=dbg_cb[b][i_][:, :, :],
                                        in_=c_blocks[nm_][:, :, :])
                nc.scalar.dma_start(out=dbg_vact[b][:, :, :],
                                    in_=vact_sb[:, :, :])

            # ---- build Qh[b][k, h*128+q'] = q_[k, b*128+q'] * h_mat[h, k]
            # one broadcast tensor-tensor per head
            qh_sb = qhp.tile([128, JC, H_OUT * NQ], bf16, tag="qh")
            qa = qact_sb[:, :, b * NQ:(b + 1) * NQ]
            for h in range(H_OUT):
                nc.vector.tensor_tensor(
                    out=qh_sb[:, :, h * NQ:(h + 1) * NQ],
                    in0=qa,
                    in1=hm_sb[:, :, h].broadcast_to([128, JC, NQ]),
                    op=MUL,
                )
            if _DEBUG:
                nc.scalar.dma_start(out=dbg_qh[b][:, :, :], in_=qh_sb[:, :, :])

            # ---- stage C: logits[b,h] = (qh_h).T @ v_ -> psum [128q, 512v]
            for h in range(H_OUT):
                last = (b == BPC - 1 and h == H_OUT - 1)
                if not last:
                    po = psC.tile([128, NV], f32, tag="psC")
                    for j in range(JC):
                        nc.tensor.matmul(
                            po,
                            lhsT=qh_sb[:, j, h * NQ:(h + 1) * NQ],
                            rhs=vact_sb[:, j, :],
                            start=(j == 0), stop=(j == JC - 1),
                        )
                    o_sb = outp.tile([128, NV], f32, tag="osb")
                    nc.scalar.activation(
                        out=o_sb, in_=po,
                        func=Ident, bias=hb_sb[:, h:h + 1], scale=1.0,
                    )
                    eng = nc.gpsimd if h % 2 == 0 else nc.sync
                    eng.dma_start(out=out[b, h, :, :], in_=o_sb)
                else:
                    # last output: two free-256 chains in separate psC tiles
                    # so the first half's act+store launches early and the
                    # final store is only 128KB
                    engs = (nc.gpsimd, nc.sync)
                    for half in range(2):
                        sl = slice(half * NH, (half + 1) * NH)
                        po = psC.tile([128, NH], f32, tag="psC",
                                      name=f"psC_last{half}")
                        for j in range(JC):
                            nc.tensor.matmul(
                                po,
                                lhsT=qh_sb[:, j, h * NQ:(h + 1) * NQ],
                                rhs=vact_sb[:, j, sl],
                                start=(j == 0), stop=(j == JC - 1),
                            )
                        o_sb = outp.tile([128, NH], f32, tag="osb",
                                         name=f"osb_last{half}")
                        nc.scalar.activation(
                            out=o_sb, in_=po,
                            func=Ident, bias=hb_sb[:, h:h + 1], scale=1.0,
                        )
                        engs[half].dma_start(out=out[b, h, :, sl], in_=o_sb)

    nc.compile()
    return nc


def kernel(v, q, Wv, bv, Wq, bq, h_mat, h_bias):
    import ml_dtypes
    from concourse import bass_utils

    bf16 = ml_dtypes.bfloat16

    if "nc" not in _CACHE:
        _CACHE["nc"] = _build_nc()
    nc = _CACHE["nc"]

    v = np.asarray(v, dtype=np.float32)
    q = np.asarray(q, dtype=np.float32)
    Wv = np.asarray(Wv, dtype=np.float32)
    Wq = np.asarray(Wq, dtype=np.float32)
    bv = np.asarray(bv, dtype=np.float32)
    bq = np.asarray(bq, dtype=np.float32)
    h_mat = np.asarray(h_mat, dtype=np.float32)
    h_bias = np.asarray(h_bias, dtype=np.float32)

    vT = np.ascontiguousarray(v.transpose(0, 2, 1)).astype(bf16)      # (B, 2048, 512)
    WvT_f = np.ascontiguousarray(Wv.T)                                # (2048, 1536) f32
    WvT = WvT_f.astype(bf16)
    WqT = np.ascontiguousarray(Wq.T).astype(bf16)                     # (1024, 1536)
    bvT = np.ascontiguousarray(bv.reshape(JC, 128).T)                 # (128, 12)
    bqT = np.ascontiguousarray(bq.reshape(JC, 128).T)
    hmP = np.ascontiguousarray(h_mat.reshape(H_OUT, JC, 128).transpose(2, 1, 0))
    hbB = np.ascontiguousarray(np.broadcast_to(h_bias[None, :], (128, H_OUT)))

    # Strassen W-combos in lhsT layout [128, 8, 768], stream order
    # [A11+A12, A11+A22, A12-A22, A21+A22, A21-A11]
    # (A11=W[0:768,0:1024] -> WvT[0:1024, 0:768] etc.)
    T = WvT_f
    combos = [
        T[0:1024, 0:768] + T[1024:2048, 0:768],     # A11+A12  (c3, M5)
        T[0:1024, 0:768] + T[1024:2048, 768:1536],  # A11+A22  (c1, M1)
        T[1024:2048, 0:768] - T[1024:2048, 768:1536],  # A12-A22 (c5, M7)
        T[0:1024, 768:1536] + T[1024:2048, 768:1536],  # A21+A22 (c2, M2)
        T[0:1024, 768:1536] - T[0:1024, 0:768],     # A21-A11  (c4, M6)
    ]
    cmbA = np.stack([
        np.ascontiguousarray(
            c.reshape(DCQ, 128, 768).transpose(1, 0, 2)).astype(bf16)
        for c in combos
    ])  # (5, 128, 8, 768)

    in_maps = []
    for c in range(N_CORES):
        bs = slice(BPC * c, BPC * (c + 1))
        qTc = np.ascontiguousarray(
            q[bs].transpose(2, 0, 1).reshape(Q_DIM, BPC * NQ)
        ).astype(bf16)
        in_maps.append({
            "vT": vT[bs],
            "qT": qTc,
            "WvT": WvT,
            "cmb": cmbA,
            "WqT": WqT,
            "bvT": bvT,
            "bqT": bqT,
            "hm": hmP,
            "hb": hbB,
        })

    res = bass_utils.run_bass_kernel_spmd(nc, in_maps, list(range(N_CORES)))
    outs = np.concatenate([res.results[c]["out"] for c in range(N_CORES)], axis=0)
    # (32, 8, 128, 512) -> (32, 8, 512, 128)
    logits = outs.transpose(0, 1, 3, 2)
    return np.ascontiguousarray(logits)


# revision 33
# speedup vs baseline: 1.2334x; 1.0830x over previous
"""Trainium2 Bass kernel for nn_BCNet: three-way low-rank bilinear net.

reference:
  v_ = relu(v @ Wv.T + bv)            # (B, NV, HK)
  q_ = relu(q @ Wq.T + bq)            # (B, NQ, HK)
  logits = einsum('hk,bvk,bqk->bhvq', h_mat, v_, q_) + h_bias

Sharding: data-parallel over batch, 4 batch items per core (8 cores).
All matmuls in bf16 with fp32 PSUM accumulation.

Schedule (evolved from trace analysis; see git-less history in logs):
  - PE runs at peak cadence (~215.7ns per 512-free matmul); all the
    optimization is in (a) reducing PE work and (b) head/tail overlap.
  - Stage A (the 12.9 GF/core v-projection) uses one level of Strassen
    over the (j=1536, k=2048) blocks of Wv: 7 products of 48 free-256
    matmuls instead of 8 block-products -> 36.6us instead of 41.4us per
    batch.  The 5 W-side block-combinations are precomputed on the host
    (free) and streamed per batch through the SBUF buffers that held Wq
    during stage B (tag-shared, zero extra SBUF).  The 5 V-side
    combinations are cheap DVE tensor-tensor ops on vt slices.  Product
    results drain from PSUM into f32 SBUF accumulators (Scalar copy for
    the first contribution, DVE adds after), then per-j-chunk
    ReLU+bias activations produce vact.  Product order (M3,M4,M5,M1,
    M7,M2,M6) completes the C-blocks in the order stage C consumes
    them, so the PE never waits at the A->C boundary.
  - Warmup junk matmuls ride out the PE p-state ramp while the first
    DMAs land; bulk loads all ride ONE queue (cold queues ramp slowly)
    in exact consumption order; stage B is an 8-wide then 4-wide pass
    so early chunk consumption stays above the cold-queue delivery rate.
  - Stage C is head-major: psum [128q x 512v] per (b,h) in two free-256
    n-half chains; h_bias[h] is a per-partition constant so the Scalar
    activation (Identity, bias) does psum->SBUF + bias in one op; the
    host transposes (q,v)->(v,q) at the end.

Host prep per core:
  vT   (4, 2048, 512) bf16  : v[b].T per batch item
  qT   (1024, 512)    bf16  : q[4c:4c+4] transposed+stacked, cols = b*128+q
  WvT  (2048, 1536)   bf16  (only the A11/A22 quadrants are DMA'd)
  cmb  (5, 128, 8, 768) bf16: Strassen W-combos, lhsT layout, stream order
                              [A11+A12, A11+A22, A12-A22, A21+A22, A21-A11]
  WqT  (1024, 1536)   bf16
  bvT  (128, 12) f32 : bv[jc*128+p]
  bqT  (128, 12) f32
  hm   (128, 12, 8) f32 : h_mat[h, jc*128+p]
  hb   (128, 8)  f32 : h_bias[h] broadcast over partitions
Device output per core: out (4, 8, 128, 512) f32 = [b, h, q, v].
Host post: concat -> (32, 8, 128, 512) -> transpose -> (32, 8, 512, 128).
"""

import numpy as np

B, NV, NQ = 32, 512, 128
V_DIM, Q_DIM, HK, H_OUT = 2048, 1024, 1536, 8
N_CORES = 8
BPC = B // N_CORES          # 4 batch items per core
JC = HK // 128              # 12 k-chunks
DCV = V_DIM // 128          # 16 contraction chunks for v
DCQ = Q_DIM // 128          # 8 contraction chunks for q
NH = NV // 2                # 256: Strassen n-half

_CACHE = {}
_DEBUG = False


def _build_nc():
    import concourse.tile as tile
    from concourse import bacc, mybir
    from contextlib import ExitStack

    bf16 = mybir.dt.bfloat16
    f32 = mybir.dt.float32
    ADD = mybir.AluOpType.add
    SUB = mybir.AluOpType.subtract
    MUL = mybir.AluOpType.mult
    Relu = mybir.ActivationFunctionType.Relu
    Ident = mybir.ActivationFunctionType.Identity
    Copy = mybir.ActivationFunctionType.Copy

    nc = bacc.Bacc()

    vT = nc.declare_dram_parameter("vT", [BPC, V_DIM, NV], bf16, isOutput=False)
    qT = nc.declare_dram_parameter("qT", [Q_DIM, BPC * NQ], bf16, isOutput=False)
    WvT = nc.declare_dram_parameter("WvT", [V_DIM, HK], bf16, isOutput=False)
    cmb = nc.declare_dram_parameter("cmb", [5, 128, DCQ, 768], bf16, isOutput=False)
    WqT = nc.declare_dram_parameter("WqT", [Q_DIM, HK], bf16, isOutput=False)
    bvT = nc.declare_dram_parameter("bvT", [128, JC], f32, isOutput=False)
    bqT = nc.declare_dram_parameter("bqT", [128, JC], f32, isOutput=False)
    hm = nc.declare_dram_parameter("hm", [128, JC, H_OUT], f32, isOutput=False)
    hb = nc.declare_dram_parameter("hb", [128, H_OUT], f32, isOutput=False)
    out = nc.declare_dram_parameter("out", [BPC, H_OUT, NQ, NV], f32, isOutput=True)
    if _DEBUG:
        dbg_vact = nc.declare_dram_parameter(
            "dbg_vact", [BPC, 128, JC, NV], bf16, isOutput=True)
        dbg_qh = nc.declare_dram_parameter(
            "dbg_qh", [BPC, 128, JC, H_OUT * NQ], bf16, isOutput=True)
        dbg_qact = nc.declare_dram_parameter(
            "dbg_qact", [128, JC, BPC * NQ], bf16, isOutput=True)
        dbg_cb = nc.declare_dram_parameter(
            "dbg_cb", [BPC, 4, 128, 6, 256], f32, isOutput=True)

    with ExitStack() as ctx:
        tc = ctx.enter_context(tile.TileContext(nc))
        consts = ctx.enter_context(tc.tile_pool(name="consts", bufs=1))
        qpool = ctx.enter_context(tc.tile_pool(name="qpool", bufs=1))
        vin = ctx.enter_context(tc.tile_pool(name="vin", bufs=2))
        vact = ctx.enter_context(tc.tile_pool(name="vact", bufs=2))
        qhp = ctx.enter_context(tc.tile_pool(name="qhp", bufs=1))
        junkp = ctx.enter_context(tc.tile_pool(name="junkp", bufs=1))
        tcomb = ctx.enter_context(tc.tile_pool(name="tcomb", bufs=3))
        csb = ctx.enter_context(tc.tile_pool(name="csb", bufs=4))
        outp = ctx.enter_context(tc.tile_pool(name="outp", bufs=3))
        psM = ctx.enter_context(tc.tile_pool(name="psM", bufs=2, space="PSUM"))
        psC = ctx.enter_context(tc.tile_pool(name="psC", bufs=2, space="PSUM"))

        # ---- SBUF destination tiles ----
        qt_sb = qpool.tile([128, DCQ, BPC * NQ], bf16)
        # wq lives in two [128,8,1024] buffers (tag "wqbig"); the Strassen
        # W-combos later rotate through the same two buffers.
        wq1_sb = consts.tile([128, DCQ, 1024], bf16, tag="wqbig", bufs=2,
                             name="wq1")
        wq2_sb = consts.tile([128, DCQ, 1024], bf16, tag="wqbig", bufs=2,
                             name="wq2")
        wv11_sb = consts.tile([128, DCQ, 768], bf16, name="wv11")
        wv22_sb = consts.tile([128, DCQ, 768], bf16, name="wv22")
        bq_sb = consts.tile([128, JC], f32)
        bv_sb = consts.tile([128, JC], f32)
        hm_sb = consts.tile([128, JC, H_OUT], f32)
        hb_sb = consts.tile([128, H_OUT], f32)
        vt0_sb = vin.tile([128, DCV, NV], bf16, tag="vt", name="vt0")

        qT_r = qT.rearrange("(d p) n -> p d n", p=128)
        WqT_r = WqT.rearrange("(d p) j -> p d j", p=128)
        WvT_r = WvT.rearrange("(d p) j -> p d j", p=128)
        vT0_r = vT[0].rearrange("(d p) n -> p d n", p=128)

        # ---- warmup: junk matmuls to ride out the PE p-state ramp while
        # the first DMAs land.
        junk = junkp.tile([128, NV], bf16)
        nc.vector.memset(junk, 0.0)
        ps_junk = psC.tile([128, NV], f32, tag="psC", name="ps_junk")
        for w in range(13):
            nc.tensor.matmul(
                ps_junk[:, 0:256], lhsT=junk[:, 0:128], rhs=junk[:, 0:256],
                start=(w == 0), stop=(w == 12),
            )

        # ---- input DMAs: tiny consts on GpSimd; all bulk loads on the
        # single Sync queue in exact consumption order.
        nc.gpsimd.dma_start(out=bq_sb, in_=bqT[:, :])
        nc.gpsimd.dma_start(out=hm_sb, in_=hm[:, :, :])
        nc.gpsimd.dma_start(out=bv_sb, in_=bvT[:, :])
        nc.gpsimd.dma_start(out=hb_sb, in_=hb[:, :])

        for d in range(DCQ):
            nc.sync.dma_start(out=qt_sb[:, d, :], in_=qT_r[:, d, :])
            nc.sync.dma_start(out=wq1_sb[:, d, :], in_=WqT_r[:, d, 0:1024])
        for d in range(DCQ):
            nc.sync.dma_start(out=wq2_sb[:, d, 0:512],
                              in_=WqT_r[:, d, 1024:HK])
        # vt0 fully (the first V-side combos need both k-halves)
        for g in range(4):
            nc.sync.dma_start(out=vt0_sb[:, 4 * g:4 * g + 4, :],
                              in_=vT0_r[:, 4 * g:4 * g + 4, :])
        # A11 / A22 quadrants of Wv, chunked for cold-queue pacing
        for d in range(DCQ):
            nc.sync.dma_start(out=wv11_sb[:, d, :], in_=WvT_r[:, d, 0:768])
        for d in range(DCQ):
            nc.sync.dma_start(out=wv22_sb[:, d, :],
                              in_=WvT_r[:, DCQ + d, 768:HK])

        # ---- stage B: q_ = relu(q @ Wq.T + bq), all 4 b at once ----
        # 8-wide pass1 / 4-wide pass2; chains live in [128,6,256] psM tiles
        # (3 x 512-wide slice-chains each) plus borrowed psC banks.
        qact_sb = qpool.tile([128, JC, BPC * NQ], bf16)

        def b_chain_targets(npsm, npsc, tagp):
            """Chain targets as (tile, kind, idx): psM tiles host 3
            512-wide slice-chains each; psC tiles host one."""
            targets = []
            for t in range(npsm):
                m = psM.tile([128, 6, NH], f32, tag="psM", name=f"{tagp}m{t}")
                for c in range(3):
                    targets.append((m, "m", c))
            for t in range(npsc):
                p = psC.tile([128, NV], f32, tag="psC", name=f"{tagp}c{t}")
                targets.append((p, "p", 0))
            return targets

        def chain_ap(tgt):
            tile_, kind, c = tgt
            return tile_[:, 2 * c:2 * c + 2, :] if kind == "m" else tile_[:, :]

        def half_ap(tgt, hf):
            tile_, kind, c = tgt
            if kind == "m":
                return tile_[:, 2 * c + hf, :]
            return tile_[:, hf * NH:(hf + 1) * NH]

        def w_slice(j, d):
            if j < 8:
                return wq1_sb[:, d, j * 128:(j + 1) * 128]
            return wq2_sb[:, d, (j - 8) * 128:(j - 8 + 1) * 128]

        def run_b_pass(j0, nj, targets):
            for d in range(DCQ - 2):
                for i in range(nj):
                    nc.tensor.matmul(
                        chain_ap(targets[i]), lhsT=w_slice(j0 + i, d),
                        rhs=qt_sb[:, d, :], start=(d == 0), stop=False,
                    )
            for i in range(nj):
                j = j0 + i
                for d in (DCQ - 2, DCQ - 1):
                    nc.tensor.matmul(
                        chain_ap(targets[i]), lhsT=w_slice(j, d),
                        rhs=qt_sb[:, d, :], start=False, stop=(d == DCQ - 1),
                    )
                for hf in range(2):
                    nc.scalar.activation(
                        out=qact_sb[:, j, hf * NH:(hf + 1) * NH],
                        in_=half_ap(targets[i], hf),
                        func=Relu, bias=bq_sb[:, j:j + 1], scale=1.0,
                    )

        run_b_pass(0, 8, b_chain_targets(2, 2, "psB1"))
        run_b_pass(8, 4, b_chain_targets(1, 1, "psB2"))
        if _DEBUG:
            nc.scalar.dma_start(out=dbg_qact[:, :, :], in_=qact_sb[:, :, :])

        # ---- per-batch: Strassen stage A, qh build, stage C ----
        # Strassen blocks (j x k): A11=Wv[0:768,0:1024] etc.
        # products in completion order needed by stage C:
        #   P1 M3 = A11 (B12-B22)      -> C12 += , C22 +=
        #   P2 M4 = A22 (B21-B11)      -> C11 += , C21 +=
        #   P3 M5 = (A11+A12) B22      -> C12 done, C11 -=
        #   P4 M1 = (A11+A22)(B11+B22) -> C11 += , C22 +=
        #   P5 M7 = (A12-A22)(B21+B22) -> C11 done
        #   P6 M2 = (A21+A22) B11      -> C21 done, C22 -=
        #   P7 M6 = (A21-A11)(B11+B12) -> C22 done
        # C11 -> vact[:,0:6,0:256], C12 -> vact[:,0:6,256:512],
        # C21 -> vact[:,6:12,0:256], C22 -> vact[:,6:12,256:512]
        for b in range(BPC):
            if b > 0:
                vt_sb = vin.tile([128, DCV, NV], bf16, tag="vt")
                vT_r = vT[b].rearrange("(d p) n -> p d n", p=128)
                nc.sync.dma_start(out=vt_sb[:, :, :], in_=vT_r[:, :, :])
            else:
                vt_sb = vt0_sb

            # stream this batch's W-combos through the wqbig buffers
            cmb_t = []
            for i in range(5):
                t = consts.tile([128, DCQ, 1024], bf16, tag="wqbig", bufs=2,
                                name=f"cmb{b}_{i}")
                nc.sync.dma_start(out=t[:, :, 0:768], in_=cmb[i][:, :, :])
                cmb_t.append(t)
            c3, c1, c5, c2, c4 = cmb_t

            # B-side slices / combos
            B11 = vt_sb[:, 0:8, 0:NH]
            B12 = vt_sb[:, 0:8, NH:NV]
            B21 = vt_sb[:, 8:16, 0:NH]
            B22 = vt_sb[:, 8:16, NH:NV]

            def tt_tile(name, src0, src1, op):
                t = tcomb.tile([128, DCQ, NH], bf16, tag="tc", name=name)
                nc.vector.tensor_tensor(out=t, in0=src0, in1=src1, op=op)
                return t

            t3 = tt_tile(f"t3_{b}", B12, B22, SUB)
            t4 = tt_tile(f"t4_{b}", B21, B11, SUB)

            vact_sb = vact.tile([128, JC, NV], bf16, tag="vact")
            c_blocks = {}
            for nm in ("C11", "C12", "C21", "C22"):
                c_blocks[nm] = csb.tile([128, 6, NH], f32, tag="csb",
                                        name=f"{nm}_{b}")

            def product(lhs_sb, lhs_off, rhs, actions, relu=None,
                        stagger=False):
                """One Strassen product.  The psum tile is zeroed once and
                all chains accumulate with start=False (start resets whole
                PSUM banks, corrupting bank-sharing neighbours), so drains
                are coarse ops per action.  actions: (dst, op, engine) with
                op in ('copy', ADD, SUB).  stagger=True runs chains
                jc-major with per-chunk drain+relu so vact chunks complete
                just ahead of stage C's consumption."""
                # start=True resets the WHOLE psum bank, so only the
                # even chain of each bank-sharing pair starts (zeroing its
                # mate's half too); the odd chain accumulates from zero.
                pm = psM.tile([128, 6, NH], f32, tag="psM")

                def chain(jc):
                    for d in range(DCQ):
                        nc.tensor.matmul(
                            pm[:, jc, :],
                            lhsT=lhs_sb[:, d, lhs_off + jc * 128:
                                        lhs_off + (jc + 1) * 128],
                            rhs=rhs[:, d, :],
                            start=(d == 0 and jc % 2 == 0),
                            stop=(d == DCQ - 1),
                            skip_group_check=True,
                        )

                def drain(jc_sl, jpos):
                    for dst, op, eng in actions:
                        cb = c_blocks[dst]
                        if op == "copy":
                            nc.scalar.activation(out=cb[:, jc_sl, :],
                                                 in_=pm[:, jc_sl, :],
                                                 func=Copy)
                        else:
                            eng.tensor_tensor(out=cb[:, jc_sl, :],
                                              in0=cb[:, jc_sl, :],
                                              in1=pm[:, jc_sl, :], op=op)
                    if relu is not None:
                        nm_, j0_, nsl_ = relu
                        for jc in jpos:
                            j = j0_ + jc
                            nc.scalar.activation(
                                out=vact_sb[:, j, nsl_],
                                in_=c_blocks[nm_][:, jc, :],
                                func=Relu, bias=bv_sb[:, j:j + 1], scale=1.0,
                            )

                if not stagger:
                    for d in range(DCQ):
                        for jc in range(6):
                            nc.tensor.matmul(
                                pm[:, jc, :],
                                lhsT=lhs_sb[:, d, lhs_off + jc * 128:
                                            lhs_off + (jc + 1) * 128],
                                rhs=rhs[:, d, :],
                                start=(d == 0 and jc % 2 == 0),
                                stop=(d == DCQ - 1),
                                skip_group_check=True,
                            )
                    drain(slice(0, 6), range(6))
                else:
                    for jc in range(6):
                        chain(jc)
                        drain(slice(jc, jc + 1), [jc])

            n1 = slice(0, NH)
            n2 = slice(NH, NV)

            # qh build emitted here so it sits EARLY in the Vector queue
            # (stage C needs it; deps only on qact + previous stage C)
            qh_sb = qhp.tile([128, JC, H_OUT * NQ], bf16, tag="qh")
            qa = qact_sb[:, :, b * NQ:(b + 1) * NQ]
            for h in range(H_OUT):
                nc.vector.tensor_tensor(
                    out=qh_sb[:, :, h * NQ:(h + 1) * NQ],
                    in0=qa,
                    in1=hm_sb[:, :, h].broadcast_to([128, JC, NQ]),
                    op=MUL,
                )

            V = nc.vector
            product(wv11_sb, 0, t3,
                    [("C12", "copy", None), ("C22", "copy", None)])       # M3
            product(wv22_sb, 0, t4,
                    [("C11", "copy", None), ("C21", "copy", None)])       # M4
            t1 = tt_tile(f"t1_{b}", B11, B22, ADD)
            product(c3, 0, B22, [("C12", ADD, V), ("C11", SUB, V)],
                    relu=("C12", 0, n2))                                  # M5
            t7 = tt_tile(f"t7_{b}", B21, B22, ADD)
            product(c1, 0, t1, [("C11", ADD, V), ("C22", ADD, V)])        # M1
            t6 = tt_tile(f"t6_{b}", B11, B12, ADD)
            product(c5, 0, t7, [("C11", ADD, V)], relu=("C11", 0, n1))    # M7
            product(c2, 0, B11, [("C21", ADD, V), ("C22", SUB, V)],
                    relu=("C21", 6, n1))                                  # M2
            product(c4, 0, t6, [("C22", ADD, V)], relu=("C22", 6, n2),
                    stagger=True)                                         # M6
            if _DEBUG:
                for i_, nm_ in enumerate(("C11", "C12", "C21", "C22")):
                    nc.gpsimd.dma_start(out=dbg_cb[b][i_][:, :, :],
                                        in_=c_blocks[nm_][:, :, :])
                nc.scalar.dma_start(out=dbg_vact[b][:, :, :],
                                    in_=vact_sb[:, :, :])

            # ---- build Qh
            # ---- stage C: logits[b,h] = (qh_h).T @ v_ -> psum [128q, 512v]
            for h in range(H_OUT):
                last = (b == BPC - 1 and h == H_OUT - 1)
                if not last:
                    po = psC.tile([128, NV], f32, tag="psC")
                    for j in range(JC):
                        nc.tensor.matmul(
                            po,
                            lhsT=qh_sb[:, j, h * NQ:(h + 1) * NQ],
                            rhs=vact_sb[:, j, :],
                            start=(j == 0), stop=(j == JC - 1),
                        )
                    o_sb = outp.tile([128, NV], f32, tag="osb")
                    nc.scalar.activation(
                        out=o_sb, in_=po,
                        func=Ident, bias=hb_sb[:, h:h + 1], scale=1.0,
                    )
                    eng = nc.gpsimd if h % 2 == 0 else nc.sync
                    eng.dma_start(out=out[b, h, :, :], in_=o_sb)
                else:
                    # last output: two free-256 chains in separate psC tiles
                    # so the first half's act+store launches early and the
                    # final store is only 128KB
                    engs = (nc.gpsimd, nc.sync)
                    for half in range(2):
                        sl = slice(half * NH, (half + 1) * NH)
                        po = psC.tile([128, NH], f32, tag="psC",
                                      name=f"psC_last{half}")
                        for j in range(JC):
                            nc.tensor.matmul(
                                po,
                                lhsT=qh_sb[:, j, h * NQ:(h + 1) * NQ],
                                rhs=vact_sb[:, j, sl],
                                start=(j == 0), stop=(j == JC - 1),
                            )
                        o_sb = outp.tile([128, NH], f32, tag="osb",
                                         name=f"osb_last{half}")
                        nc.scalar.activation(
                            out=o_sb, in_=po,
                            func=Ident, bias=hb_sb[:, h:h + 1], scale=1.0,
                        )
                        engs[half].dma_start(out=out[b, h, :, sl], in_=o_sb)

    nc.compile()
    return nc


def kernel(v, q, Wv, bv, Wq, bq, h_mat, h_bias):
    import ml_dtypes
    from concourse import bass_utils

    bf16 = ml_dtypes.bfloat16

    if "nc" not in _CACHE:
        _CACHE["nc"] = _build_nc()
    nc = _CACHE["nc"]

    v = np.asarray(v, dtype=np.float32)
    q = np.asarray(q, dtype=np.float32)
    Wv = np.asarray(Wv, dtype=np.float32)
    Wq = np.asarray(Wq, dtype=np.float32)
    bv = np.asarray(bv, dtype=np.float32)
    bq = np.asarray(bq, dtype=np.float32)
    h_mat = np.asarray(h_mat, dtype=np.float32)
    h_bias = np.asarray(h_bias, dtype=np.float32)

    vT = np.ascontiguousarray(v.transpose(0, 2, 1)).astype(bf16)      # (B, 2048, 512)
    WvT_f = np.ascontiguousarray(Wv.T)                                # (2048, 1536) f32
    WvT = WvT_f.astype(bf16)
    WqT = np.ascontiguousarray(Wq.T).astype(bf16)                     # (1024, 1536)
    bvT = np.ascontiguousarray(bv.reshape(JC, 128).T)                 # (128, 12)
    bqT = np.ascontiguousarray(bq.reshape(JC, 128).T)
    hmP = np.ascontiguousarray(h_mat.reshape(H_OUT, JC, 128).transpose(2, 1, 0))
    hbB = np.ascontiguousarray(np.broadcast_to(h_bias[None, :], (128, H_OUT)))

    # Strassen W-combos in lhsT layout [128, 8, 768], stream order
    # [A11+A12, A11+A22, A12-A22, A21+A22, A21-A11]
    # (A11=W[0:768,0:1024] -> WvT[0:1024, 0:768] etc.)
    T = WvT_f
    combos = [
        T[0:1024, 0:768] + T[1024:2048, 0:768],     # A11+A12  (c3, M5)
        T[0:1024, 0:768] + T[1024:2048, 768:1536],  # A11+A22  (c1, M1)
        T[1024:2048, 0:768] - T[1024:2048, 768:1536],  # A12-A22 (c5, M7)
        T[0:1024, 768:1536] + T[1024:2048, 768:1536],  # A21+A22 (c2, M2)
        T[0:1024, 768:1536] - T[0:1024, 0:768],     # A21-A11  (c4, M6)
    ]
    cmbA = np.stack([
        np.ascontiguousarray(
            c.reshape(DCQ, 128, 768).transpose(1, 0, 2)).astype(bf16)
        for c in combos
    ])  # (5, 128, 8, 768)

    in_maps = []
    for c in range(N_CORES):
        bs = slice(BPC * c, BPC * (c + 1))
        qTc = np.ascontiguousarray(
            q[bs].transpose(2, 0, 1).reshape(Q_DIM, BPC * NQ)
        ).astype(bf16)
        in_maps.append({
            "vT": vT[bs],
            "qT": qTc,
            "WvT": WvT,
            "cmb": cmbA,
            "WqT": WqT,
            "bvT": bvT,
            "bqT": bqT,
            "hm": hmP,
            "hb": hbB,
        })

    res = bass_utils.run_bass_kernel_spmd(nc, in_maps, list(range(N_CORES)))
    outs = np.concatenate([res.results[c]["out"] for c in range(N_CORES)], axis=0)
    # (32, 8, 128, 512) -> (32, 8, 512, 128)
    logits = outs.transpose(0, 1, 3, 2)
    return np.ascontiguousarray(logits)
